# revision 20
# baseline (speedup 1.0000x reference)
"""Multi-head attention Trainium2 Bass kernel.

Problem: B=2, T=2048, D=1024, H=16 heads, head dim K=64.
Sharding: 8 cores = 2 batches x 4 head-groups (4 heads each).
Each core computes q/k/v projections for its head slice, attention for its
4 heads, and a partial output projection; host sums partials over head
groups and adds the output bias.

v4: ACT-bound design (exp over the full S matrix = 16.8M elem/core ~=
147us at 1 elem/cyc/lane is the floor), everything else hides under it:
 - x is shipped host-side pre-transposed + bf16 ([128p(d), qc, dt, 512t]),
   weights host-pre-slabbed bf16: no on-chip transposes/casts, 6.6MB
   total input DMA split across the three rings as ~1MB transfers.
 - q/k are fp16 (10-bit mantissa, |q|<~10): the S matmul moving operand
   streams 1 col/cyc (f32r ran in half-rate fp32 mode), softmax P and V
   fp16.
 - attention is software-pipelined across the 8 (qc, head-pair)
   iterations: the next iteration's first two S/exp are emitted before
   the previous tail, PE filler work (V/K/Q projections in iteration 0,
   out-proj chunks later) is spread one closure per k-tile, so ACT never
   idles from ~12us until the last exp.

Layouts per core:
  xT      [128p, NQ, ND, 512] bf16   x^T, d on partitions (host layout)
  qt/kt   [128p, 2, T] fp16    head 2j at p 0:64 of slab j, 2j+1 at 64:128
  v_aug   [128p, NT, 4, 65] fp16  V natural per k-tile/head + ones col
  S^T     [k=128, q] psum f32; exp'd to fp16 pp (ACT)
  PV      pp chunk stationary, v_aug moving -> attention-natural
          [128q, 65] with sumexp in col 64; normalize = reciprocal +
          tensor_scalar_mul; PE-transpose back to attnT for out proj
"""

from collections import deque
from contextlib import ExitStack

import numpy as np

import concourse.bass as bass
import concourse.tile as tile
from concourse import bacc, mybir
from concourse.masks import make_identity

F32 = mybir.dt.float32
BF16 = mybir.dt.bfloat16
FP16 = mybir.dt.float16

# problem config (hardcoded per contest rules)
B, T, D = 2, 2048, 1024
H, HK = 16, 64          # total heads, head dim
NCORES = 8
HPC = H // (NCORES // B)   # heads per core = 4
DH = HPC * HK              # per-core hk slice width = 256
QC = 512                   # free-dim chunk for matmuls


def body(tc, outs, ins, cfg, reps=1):
    nc = tc.nc
    T_, D_, DH_ = cfg["T"], cfg["D"], cfg["DH"]
    QC_ = cfg["QC"]
    NT = T_ // 128          # token tiles (= k tiles)
    ND = D_ // 128          # d tiles
    NQ = T_ // QC_          # q chunks
    NHT = DH_ // 128        # hk partition slabs (= head pairs)
    NKT = NT
    NPQ = QC_ // 128
    NOC = D_ // QC_

    xt_in, wq, wk, wv, wo, bq, bk, bv = (
        ins["xt"], ins["wq"], ins["wk"], ins["wv"], ins["wo"],
        ins["bq"], ins["bk"], ins["bv"],
    )
    out = outs["out"]

    with ExitStack() as ctx:
        # ---- persistent SBUF tensors ----
        xT = nc.alloc_sbuf_tensor("xT", [128, NQ, ND, QC_], BF16).ap()
        qt = nc.alloc_sbuf_tensor("qt", [128, NHT, T_], FP16).ap()
        kt = nc.alloc_sbuf_tensor("kt", [128, NHT, T_], FP16).ap()
        v_aug = nc.alloc_sbuf_tensor("v_aug", [128, NKT, HPC, HK + 1], FP16).ap()
        wqb = nc.alloc_sbuf_tensor("wqb", [128, ND, DH_], BF16).ap()
        wkb = nc.alloc_sbuf_tensor("wkb", [128, ND, DH_], BF16).ap()
        wvb = nc.alloc_sbuf_tensor("wvb", [128, ND, DH_], BF16).ap()
        wob = nc.alloc_sbuf_tensor("wob", [128, NHT, D_], BF16).ap()
        ident = nc.alloc_sbuf_tensor("ident", [128, 128], F32).ap()
        ident_bf = nc.alloc_sbuf_tensor("ident_bf", [128, 128], BF16).ap()
        bq_sb = nc.alloc_sbuf_tensor("bq_sb", [128, NHT], F32).ap()
        bk_sb = nc.alloc_sbuf_tensor("bk_sb", [128, NHT], F32).ap()
        bv_row = nc.alloc_sbuf_tensor("bv_row", [1, DH_], F32).ap()
        bv_bc = nc.alloc_sbuf_tensor("bv_bc", [128, DH_], F32).ap()
        ones_f32 = nc.alloc_sbuf_tensor("ones_f32", [128, NKT * HPC], F32).ap()

        # ---- pools ----
        ptil = ctx.enter_context(tc.tile_pool(name="ptil", bufs=6))
        atp = ctx.enter_context(tc.tile_pool(name="atp", bufs=2))
        outp = ctx.enter_context(tc.tile_pool(name="outp", bufs=2))
        natp = ctx.enter_context(tc.tile_pool(name="natp", bufs=4))
        rzp = ctx.enter_context(tc.tile_pool(name="rzp", bufs=8))
        psA = ctx.enter_context(tc.tile_pool(name="psA", bufs=2, space="PSUM"))
        psS = ctx.enter_context(tc.tile_pool(name="psS", bufs=2, space="PSUM"))
        psO = ctx.enter_context(tc.tile_pool(name="psO", bufs=2, space="PSUM"))

        make_identity(nc, ident)
        nc.vector.tensor_copy(ident_bf[:, :], ident[:, :])

        rings = [nc.sync, nc.gpsimd]

        for _rep in range(reps):
            # ---- input DMA, issue order = first-use order per ring.
            # Everything is bf16 and lands directly (no staging/cast).
            # Biases ride gpsimd AFTER wk (first needed ~25us).
            nc.scalar.dma_start(wvb[:, :, :], wv[:, :, :])
            nc.gpsimd.dma_start(wkb[:, :, :], wk[:, :, :])
            nc.sync.dma_start(xT[:, 0, :, :], xt_in[:, 0, :, :])
            nc.scalar.dma_start(wqb[:, :, :], wq[:, :, :])
            nc.gpsimd.dma_start(xT[:, 1, :, :], xt_in[:, 1, :, :])
            nc.sync.dma_start(xT[:, 2, :, :], xt_in[:, 2, :, :])
            nc.scalar.dma_start(xT[:, 3, :, :], xt_in[:, 3, :, :])
            for t in range(NHT):
                nc.gpsimd.dma_start(bq_sb[:, t:t + 1],
                                    bq[t * 128:(t + 1) * 128].unsqueeze(1))
                nc.gpsimd.dma_start(bk_sb[:, t:t + 1],
                                    bk[t * 128:(t + 1) * 128].unsqueeze(1))
            nc.gpsimd.dma_start(bv_row[:, :], bv.unsqueeze(0))
            nc.gpsimd.partition_broadcast(bv_bc[:, :], bv_row[:, :])
            nc.gpsimd.dma_start(wob[:, :, :], wo[:, :, :])

            nc.vector.memset(ones_f32[:, :], 1.0)
            nc.vector.tensor_copy(
                v_aug[:, :, :, HK:HK + 1],
                ones_f32.rearrange("p (n h) -> p n h", h=HPC).unsqueeze(3),
            )

            def xts(n):
                # xT slice for token tile n: [128, ND, 128]
                return xT[:, n // 4, :, (n % 4) * 128:(n % 4 + 1) * 128]

            # V for token tile n (natural layout, + bias into v_aug)
            def v_tile(n):
                xs = xts(n)
                ps = psA.tile([128, DH_], F32, tag="psA")
                for dt in range(ND):
                    nc.tensor.matmul(
                        ps[:, :], xs[:, dt, :], wvb[:, dt, :],
                        start=(dt == 0), stop=(dt == ND - 1),
                    )
                nc.vector.tensor_add(
                    v_aug[:, n, :, 0:HK],
                    ps.rearrange("p (h e) -> p h e", h=HPC),
                    bv_bc.rearrange("p (h e) -> p h e", h=HPC),
                )

            # K^T for token tiles 2c, 2c+1 (stationary = w slab)
            def k_chunk(c):
                xs = xT[:, c // 2, :, (c % 2) * 256:(c % 2) * 256 + 256]
                for s in range(NHT):
                    ps = psA.tile([128, 256], F32, tag="psA")
                    for dt in range(ND):
                        nc.tensor.matmul(
                            ps[:, :], wkb[:, dt, s * 128:(s + 1) * 128],
                            xs[:, dt, :],
                            start=(dt == 0), stop=(dt == ND - 1),
                        )
                    nc.vector.tensor_scalar_add(
                        kt[:, s, c * 256:(c + 1) * 256], ps[:, :],
                        bk_sb[:, s:s + 1])

            def q_chunk_slab(qcc, s):
                ps = psA.tile([128, QC_], F32, tag="psA")
                for dt in range(ND):
                    nc.tensor.matmul(
                        ps[:, :], wqb[:, dt, s * 128:(s + 1) * 128],
                        xT[:, qcc, dt, :],
                        start=(dt == 0), stop=(dt == ND - 1),
                    )
                nc.vector.tensor_scalar_add(
                    qt[:, s, qcc * QC_:(qcc + 1) * QC_], ps[:, :],
                    bq_sb[:, s:s + 1])

            # ---- attention, software-pipelined across (qc, hp) ----
            def emit_s_exp(st, ktt):
                ksl = slice(ktt * 128, (ktt + 1) * 128)
                qsl = st["qsl"]
                hp = st["hp"]
                sp = psS.tile([128, 2 * QC_], F32, tag="psS")
                nc.tensor.matmul(sp[:, 0:QC_], kt[0:64, hp, ksl],
                                 qt[0:64, hp, qsl],
                                 start=True, stop=True, tile_position=(0, 0))
                nc.tensor.matmul(sp[:, QC_:2 * QC_], kt[64:128, hp, ksl],
                                 qt[64:128, hp, qsl],
                                 start=True, stop=True, tile_position=(64, 0))
                pp = ptil.tile([128, 2 * QC_], FP16, tag="ptil")
                nc.scalar.activation(pp[:, :], sp[:, :],
                                     mybir.ActivationFunctionType.Exp,
                                     scale=float(1.0 / np.sqrt(HK)))
                st["pps"].append(pp)

            def acc_ap(st, hl, npq):
                po = st["po"][hl]
                return po[:, npq * 65:npq * 65 + HK + 1]

            def emit_pv(st, ktt):
                # PSUM start=True zeroes the whole bank; only the first
                # slice written in each bank (npq == 0) carries it.
                pp = st["pps"][ktt]
                hp = st["hp"]
                for hl in range(2):
                    for npq in range(NPQ):
                        nc.tensor.matmul(
                            acc_ap(st, hl, npq),
                            pp[:, hl * QC_ + npq * 128:
                               hl * QC_ + (npq + 1) * 128],
                            v_aug[:, ktt, 2 * hp + hl, :],
                            start=(ktt == 0) and npq == 0,
                            stop=(ktt == NKT - 1),
                            skip_group_check=True,
                        )

            def attention_start(qcc, hp):
                st = {"qcc": qcc, "hp": hp,
                      "qsl": slice(qcc * QC_, (qcc + 1) * QC_),
                      "pps": [], "po": None}
                emit_s_exp(st, 0)
                emit_s_exp(st, 1)
                return st

            def attention_rest(st, fillers, first_slot=4):
                # fillers run AFTER S/PV of their k-tile, and (except for
                # iteration 0) only from first_slot on, keeping the
                # iteration-boundary PE slots free so ACT never starves.
                st["po"] = (psO.tile([128, 512], F32, tag="psO", name="po0"),
                            psO.tile([128, 512], F32, tag="psO", name="po1"))
                for ktt in range(2, NKT):
                    emit_s_exp(st, ktt)
                    emit_pv(st, ktt - 2)
                    if fillers and ktt >= first_slot:
                        fillers.popleft()()

            def attention_tail(st, a_qc):
                emit_pv(st, NKT - 2)
                emit_pv(st, NKT - 1)
                hp = st["hp"]
                nats = []
                for npq in range(NPQ):
                    nat = natp.tile([128, 2 * HK], BF16, tag="natp")
                    for hl in range(2):
                        acc = acc_ap(st, hl, npq)
                        rz = rzp.tile([128, 1], F32, tag="rzp")
                        nc.vector.reciprocal(rz[:, :], acc[:, HK:HK + 1])
                        nc.vector.tensor_scalar_mul(
                            nat[:, hl * HK:(hl + 1) * HK],
                            acc[:, 0:HK], rz[:, :])
                    nats.append(nat)
                tpn = psA.tile([128, NPQ, 128], BF16, tag="psA")
                for npq in range(NPQ):
                    nc.tensor.transpose(tpn[:, npq, :], nats[npq][:, :],
                                        ident_bf[:, :])
                nc.vector.tensor_copy(
                    a_qc[:, hp, :],
                    tpn.rearrange("p a b -> p (a b)"))

            def out_proj_chunk(qcc, a_qc, npq, oc):
                n = qcc * NPQ + npq
                ps = psA.tile([128, QC_], F32, tag="psA")
                for hp in range(NHT):
                    nc.tensor.matmul(
                        ps[:, :],
                        a_qc[:, hp, npq * 128:(npq + 1) * 128],
                        wob[:, hp, oc * QC_:(oc + 1) * QC_],
                        start=(hp == 0), stop=(hp == NHT - 1),
                    )
                ot = outp.tile([128, QC_], F32, tag="outp")
                nc.vector.tensor_copy(ot[:, :], ps[:, :])
                rings[(npq + oc) % 2].dma_start(
                    out[n * 128:(n + 1) * 128, oc * QC_:(oc + 1) * QC_],
                    ot[:, :],
                )

            # ---- prologue: V(0-3), K chunks 0-1, Q(0) ----
            for n in range(4):
                v_tile(n)
                if n % 2 == 1:
                    k_chunk(n // 2)
            q_chunk_slab(0, 0)
            q_chunk_slab(0, 1)

            # iteration-0 fillers: remaining V tiles + K chunks, paced so
            # S(ktt) sees its K chunk and PV(ktt-2) its V tile in time.
            def iter0_fillers():
                fl = deque()
                for j in range(2, NKT):
                    if j % 2 == 0:
                        p = j + 2

                        def mkv(p=p):
                            def f():
                                if p <= NT - 2:
                                    v_tile(p)
                                    v_tile(p + 1)
                            return f
                        fl.append(mkv())
                    else:
                        c = (j + 1) // 2

                        def mkk(c=c):
                            def f():
                                if c <= NT // 2 - 1:
                                    k_chunk(c)
                            return f
                        fl.append(mkk())
                return fl

            its = [(qcc, hp) for qcc in range(NQ) for hp in range(NHT)]
            a_qcs = {}
            fillers = deque()
            prev = None
            for idx, (qcc, hp) in enumerate(its):
                if hp == 0:
                    a_qcs[qcc] = atp.tile([128, NHT, QC_], BF16, tag="atp",
                                          name="a_qc")
                st = attention_start(qcc, hp)
                if prev is not None:
                    attention_tail(prev, a_qcs[prev["qcc"]])
                    if prev["hp"] == 1 and prev["qcc"] < NQ - 1:
                        pq = prev["qcc"]
                        for npq in range(NPQ):
                            for oc in range(NOC):
                                fillers.append(
                                    lambda pq=pq, npq=npq, oc=oc:
                                    out_proj_chunk(pq, a_qcs[pq], npq, oc))
                if idx == 0:
                    attention_rest(st, iter0_fillers(), first_slot=2)
                else:
                    attention_rest(st, fillers)
                if idx < NQ - 1:
                    for s in range(NHT):
                        fillers.append(
                            lambda qn=idx + 1, s=s: q_chunk_slab(qn, s))
                prev = st
            attention_tail(prev, a_qcs[prev["qcc"]])
            for npq in range(NPQ):
                for oc in range(NOC):
                    out_proj_chunk(NQ - 1, a_qcs[NQ - 1], npq, oc)


def build(cfg, reps=1):
    nc = bacc.Bacc("TRN2", target_bir_lowering=False, debug=False,
                   num_devices=NCORES)
    T_, D_, DH_ = cfg["T"], cfg["D"], cfg["DH"]
    QC_ = cfg["QC"]
    ND_, NHT_, NQ_ = D_ // 128, DH_ // 128, T_ // QC_
    ins = {
        # x host-pre-transposed to [p, qc, dt, t'] bf16
        "xt": nc.dram_tensor("xt", [128, NQ_, ND_, QC_], BF16,
                             kind="ExternalInput").ap(),
        # weights host-pre-slabbed bf16: [partition, slab, cols]
        "wq": nc.dram_tensor("wq", [128, ND_, DH_], BF16,
                             kind="ExternalInput").ap(),
        "wk": nc.dram_tensor("wk", [128, ND_, DH_], BF16,
                             kind="ExternalInput").ap(),
        "wv": nc.dram_tensor("wv", [128, ND_, DH_], BF16,
                             kind="ExternalInput").ap(),
        "wo": nc.dram_tensor("wo", [128, NHT_, D_], BF16,
                             kind="ExternalInput").ap(),
        "bq": nc.dram_tensor("bq", [DH_], F32, kind="ExternalInput").ap(),
        "bk": nc.dram_tensor("bk", [DH_], F32, kind="ExternalInput").ap(),
        "bv": nc.dram_tensor("bv", [DH_], F32, kind="ExternalInput").ap(),
    }
    outs = {
        "out": nc.dram_tensor("out", [T_, D_], F32, kind="ExternalOutput").ap(),
    }
    with tile.TileContext(nc) as tc:
        body(tc, outs, ins, cfg, reps=reps)
    nc.compile()
    return nc


_NC_CACHE = {}


def _get_nc(reps=1):
    key = (T, D, DH, reps)
    if key not in _NC_CACHE:
        _NC_CACHE[key] = build({"T": T, "D": D, "DH": DH, "QC": QC}, reps=reps)
    return _NC_CACHE[key]


def _bf16():
    import ml_dtypes
    return ml_dtypes.bfloat16


def _preslab(w):
    """[nd*128, cols] -> [128, nd, cols] bf16 (slab dt = rows dt*128..+127),
    so each SBUF partition's free data is contiguous in DRAM."""
    nd = w.shape[0] // 128
    return np.ascontiguousarray(
        np.asarray(w, dtype=np.float32).reshape(nd, 128, w.shape[1])
        .transpose(1, 0, 2).astype(_bf16()))


def _prextr(x):
    """[T, D] -> [128, NQ, ND, QC] bf16 with
    out[p, c, dt, t'] = x[c*QC + t', dt*128 + p]."""
    nq, nd = T // QC, D // 128
    return np.ascontiguousarray(
        np.asarray(x, dtype=np.float32).reshape(nq, QC, nd, 128)
        .transpose(3, 0, 2, 1).astype(_bf16()))


def make_in_maps(x_q, Wq, bq, Wk, bk, Wv, bv, Wo, bo):
    in_maps = []
    for c in range(NCORES):
        b, hg = divmod(c, NCORES // B)
        sl = slice(hg * DH, (hg + 1) * DH)
        in_maps.append({
            "xt": _prextr(x_q[b]),
            "wq": _preslab(Wq[:, sl]),
            "wk": _preslab(Wk[:, sl]),
            "wv": _preslab(Wv[:, sl]),
            "wo": _preslab(Wo[sl, :]),
            "bq": np.ascontiguousarray(bq[sl], dtype=np.float32),
            "bk": np.ascontiguousarray(bk[sl], dtype=np.float32),
            "bv": np.ascontiguousarray(bv[sl], dtype=np.float32),
        })
    return in_maps


def gather(results, bo):
    ngrp = NCORES // B
    out = np.empty((B, T, D), dtype=np.float32)
    for b in range(B):
        acc = results[b * ngrp]["out"].astype(np.float32).copy()
        for hg in range(1, ngrp):
            acc += results[b * ngrp + hg]["out"]
        out[b] = acc + np.asarray(bo, dtype=np.float32)[None, :]
    return out


def kernel(x_q, Wq, bq, Wk, bk, Wv, bv, Wo, bo, _spmd_kwargs=None, _reps=1):
    from concourse.bass_utils import run_bass_kernel_spmd

    nc = _get_nc(reps=_reps)
    in_maps = make_in_maps(x_q, Wq, bq, Wk, bk, Wv, bv, Wo, bo)
    kw = _spmd_kwargs or {}
    res = run_bass_kernel_spmd(nc, in_maps, core_ids=list(range(NCORES)), **kw)
    out = gather(res.results, bo)
    kernel.last_results = res
    return out
